# revision 40
# baseline (speedup 1.0000x reference)
"""Trainium2 Bass kernel for nn_DiffKS (differentiable Karplus-Strong string).

Math:  y[t] = x[t] - sum_j vals[t,j] * y[t-1-z[t]-j],  z in [~289, ~517]
where x is the order-1-shaped excitation and vals/z come from a cubic-spline
upsampled delay/coefficient trajectory.

The feedback reaches >= ~290 samples back, so 128-sample blocks have no
intra-block dependency.  345 serial rounds; per round the 7-tap band (source
window ~134 wide, spanning 2 history columns, rarely 3) is evaluated as
exactly 2 (rarely 3) matmul pieces:

  piece A: K-range [a0,128) of history column c   (a0 in {0,64,96}, weights
           zero-padded below the first real source row)
  piece B: K-range [0,rB)   of history column c+1
  piece C: K-range [0,rC)   of history column c+2 (only ~4% of rounds;
           weights stored in the A-block's unused low rows)

All pieces use M=128 and PE-legal tile positions, PSUM-accumulated.  The
per-round chain is then just matmuls -> one DVE subtract that writes the new
fp16 history column directly (y = x - acc, cast on write).  No gpsimd cast,
no on-device output transpose: history is one [128, NCOLS] fp16 SBUF tile
(Tile hazards are range-granular, so per-column views don't false-serialize),
DMA'd out once; the host reassembles the final f32 signal.

The V tiles stream from DRAM in a partition-major layout ([128, NR, VW]) so
each partition reads contiguous multi-KB runs -- with the round-major layout
the DMA queues run at ~130 GB/s on 512B packets and gate the whole kernel.

Host does only the O(frames) spline prep, the integer structure plan, and
the (tiny) order-1 excitation scan.  Measured: ~94 us (baseline 144 us),
steady state is Tensor-engine LDWEIGHTS-throughput-bound (~102 ns per piece,
2.0 pieces/round, 96% PE occupancy); the serial feedback chain
(MM drain 173ns -> DVE 158ns, dependency distance ~2.27 rounds) sits just
under the PE throughput limit.
"""
import numpy as np
import ml_dtypes

import concourse.bacc as bacc
import concourse.mybir as mybir
from concourse.tile import TileContext
from concourse.bass_utils import run_bass_kernel_spmd

T = 44100
NFRAMES = 100
NCOEF = 6
B = 128
NR = (T + B - 1) // B          # 345 rounds
TP = NR * B                    # 44160
OFFC = 5                       # leading zero history columns
NCOLS = NR + OFFC              # 350
GRP = 16                       # V streaming group size
F32 = mybir.dt.float32
FP16 = mybir.dt.float16
NPH = 8                        # history phase tiles
SLOTS = (NCOLS + NPH - 1) // NPH   # 44
BOFF = 128                     # B block free-col offset
VW = 256                       # packed V tile free width (A cols 0..127, B cols 128..255)


TRACE = False
LAST_EXEC_NS = None
LAST_RES = None


# ----------------------------------------------------------------- host math
def _sigmoid(v):
    return 1.0 / (1.0 + np.exp(-v))


def _spline_eval(y, n_out):
    """Natural cubic spline on uniform knots in [0,1] (float64; the f32
    reference differs by ~1e-7 relative)."""
    n, d = y.shape
    h = 1.0 / (n - 1)
    rhs = 6.0 * (y[2:] - 2.0 * y[1:-1] + y[:-2]) / h
    Tm = (np.diag(np.full(n - 2, 4.0 * h))
          + np.diag(np.full(n - 3, h), 1)
          + np.diag(np.full(n - 3, h), -1))
    M_in = np.linalg.solve(Tm, rhs)
    M = np.concatenate([np.zeros((1, d)), M_in, np.zeros((1, d))])
    t_out = np.linspace(0.0, 1.0, n_out)
    idx = np.clip((t_out / h).astype(np.int32), 0, n - 2)
    f = (t_out - idx.astype(np.float64) * h)[:, None]
    y0, y1 = y[idx], y[idx + 1]
    M0, M1 = M[idx], M[idx + 1]
    b = (y1 - y0) / h - h * (2.0 * M0 + M1) / 6.0
    c = 0.5 * M0
    dd = (M1 - M0) / (6.0 * h)
    return y0 + f * (b + f * (c + f * dd))


def _host_structure(delay_len_frames, raw_gain, raw_coeff_frames):
    gain = _sigmoid(np.float64(raw_gain))
    sig = _sigmoid(np.float64(raw_coeff_frames))
    bf = sig / sig.sum(-1, keepdims=True) * gain
    params = np.concatenate([np.float64(delay_len_frames)[:, None], bf], axis=1)
    up = _spline_eval(params, T)
    delay, b = up[:, 0], up[:, 1:]
    z = np.floor(delay).astype(np.int64)
    alfa = delay - np.floor(delay)
    first = (-(1.0 - alfa) * b[:, 0])[:, None]
    mid = -(alfa[:, None] * b[:, :-1] + (1.0 - alfa)[:, None] * b[:, 1:])
    last = (-alfa * b[:, -1])[:, None]
    vals = np.concatenate([first, mid, last], axis=1)
    vf = vals[:, ::-1].copy()          # vf[t, jj] multiplies y[t-7-z[t]+jj]
    s0 = np.arange(T) - 7 - z
    return vf, s0


def _lpc1(e, a):
    """y[t] = e[t] - a[t]*y[t-1], vectorized by log-doubling in float64."""
    n = len(e)
    y = np.array(e, np.float64)
    c = -np.array(a, np.float64)       # y[t] = c[t]*y[t-1] + e[t]
    lag = 1
    while lag < n:
        y[lag:] = y[lag:] + c[lag:] * y[:-lag]
        c = np.concatenate([c[:lag] * 0.0, c[lag:] * c[:-lag]])
        lag *= 2
    return y


# ------------------------------------------------------------ blocked plan
def _build_plan(vf, s0):
    """Per-round packed V tiles [NR,128,VW] and piece lists."""
    s0p = np.concatenate([s0, s0[-1] + 1 + np.arange(TP - T)])
    vfp = np.concatenate([vf, np.zeros((TP - T, 7))]).astype(np.float64)
    soff = s0p + OFFC * B
    assert soff.min() >= 0
    vtiles = np.zeros((NR, B, VW), np.float64)
    plan = []
    for k in range(NR):
        t0 = k * B
        sl = soff[t0:t0 + B]
        smin = int(sl.min())
        smax = int(sl.max()) + 6
        c, r0 = divmod(smin, B)
        rB = smax - B * (c + 1) + 1
        rBc = min(max(rB, 0), B)
        rC = rB - B if rB > B else 0
        if rC:
            assert rC < r0, (k, rC, r0)
        a0 = 96 if r0 >= 96 else (64 if r0 >= 64 else 0)
        vt = vtiles[k]
        used = np.zeros((B, VW), np.bool_)
        for tt in range(B):
            base = int(sl[tt])
            for jj in range(7):
                s = base + jj
                row = s - B * c
                w = vfp[t0 + tt, jj]
                if row < B:
                    rr, cc = row, tt                # A block (col c)
                elif row < 2 * B:
                    rr, cc = row - B, BOFF + tt     # B block (col c+1)
                else:
                    assert row - 2 * B < r0
                    rr, cc = row - 2 * B, tt        # C in A block's low rows
                assert not used[rr, cc], (k, rr, cc)
                used[rr, cc] = True
                vt[rr, cc] = w
        pieces = []
        if c >= OFFC:                  # cols < OFFC are all-zero history
            pieces.append((a0, B, c, 0))
        if rBc and c + 1 >= OFFC:
            pieces.append((0, rBc, c + 1, BOFF))
        if rC and c + 2 >= OFFC:
            pieces.append((0, rC, c + 2, 0))
        plan.append(pieces)
    return plan, vtiles


# ------------------------------------------------------------- device build
def _build_kernel(plan):
    nc = bacc.Bacc("TRN2", target_bir_lowering=False, debug=False)
    # partition-major V layout: per-partition contiguous GRP*VW fp16 runs
    v_d = nc.dram_tensor("vtiles", [B, NR, VW], FP16, kind="ExternalInput")
    x_d = nc.dram_tensor("xcols", [B, NR], F32, kind="ExternalInput")
    y_d = nc.dram_tensor("y16", [B, NCOLS], FP16, kind="ExternalOutput")

    with TileContext(nc) as tc:
        with (
            tc.tile_pool(name="vpool", bufs=5) as vpool,
            tc.tile_pool(name="hpool", bufs=1) as hpool,
            tc.tile_pool(name="xpool", bufs=1) as xpool,
            tc.tile_pool(name="ps", bufs=6, space="PSUM") as ps,
        ):
            xt = xpool.tile([B, NR], F32)
            nc.sync.dma_start(xt[:, 0:32], x_d[:, 0:32])
            nc.scalar.dma_start(xt[:, 32:NR], x_d[:, 32:NR])
            ht = hpool.tile([B, NCOLS], FP16, tag="h", name="hist")
            nc.vector.memset(ht[:, 0:OFFC], 0.0)

            vtile = None
            for k in range(NR):
                g, kk = k // GRP, k % GRP
                if kk == 0:
                    gn = min(GRP, NR - g * GRP)
                    vtile = vpool.tile([B, GRP, VW], FP16, tag="v", name=f"v{g}")
                    g0 = g * GRP
                    if g == 0:
                        # split the first group finely across both queues:
                        # round 0 starts as soon as its own slice lands
                        splits = [(0, 1), (1, 2), (2, 4), (4, 8)]
                        if GRP > 8:
                            splits.append((8, GRP))
                        for h, (lo, hi) in enumerate(splits):
                            eng = nc.sync if (h % 2 == 0) else nc.scalar
                            eng.dma_start(
                                vtile[:, lo:hi, :], v_d[:, lo:hi, :])
                    elif g in (1, 2):
                        # row-trimmed halves for the warm-up groups: the
                        # queues can't sustain full tiles this early
                        rounds = range(g0, g0 + gn)
                        a0g = min(plan[r][0][0] for r in rounds
                                  if plan[r]) if any(plan[r] for r in rounds) else 0
                        rBg = max((p[1] for r in rounds for p in plan[r][1:2]
                                   if p[3] == BOFF), default=B)
                        gC = max((p[1] for r in rounds for p in plan[r][2:3]),
                                 default=0)
                        if gC and a0g < gC:
                            a0g = 0
                        hgn = gn // 2
                        for (e1, lo, hi) in ((nc.scalar, 0, hgn),
                                             (nc.sync, hgn, gn)):
                            e1.dma_start(
                                vtile[a0g:B, lo:hi, 0:B],
                                v_d[a0g:B, g0 + lo:g0 + hi, 0:B])
                            e1.dma_start(
                                vtile[0:rBg, lo:hi, B:VW],
                                v_d[0:rBg, g0 + lo:g0 + hi, B:VW])
                            if gC and a0g >= gC:
                                e1.dma_start(
                                    vtile[0:gC, lo:hi, 0:B],
                                    v_d[0:gC, g0 + lo:g0 + hi, 0:B])
                    else:
                        eng = nc.sync if (g % 2 == 0) else nc.scalar
                        eng.dma_start(
                            vtile[:, 0:gn, :],
                            v_d[:, g0:g0 + gn, :])
                pieces = plan[k]
                dst = k + OFFC
                hcol = ht[:, dst:dst + 1]
                if not pieces:
                    # all sources in zero history: y = x
                    nc.vector.tensor_copy(hcol, xt[:, k:k + 1])
                    continue
                acc = ps.tile([B, 1], F32, tag="acc", name=f"acc{k}")
                last = len(pieces) - 1
                for i, (kb0, kb1, col, fb) in enumerate(pieces):
                    nc.tensor.matmul(
                        acc[:, :],
                        vtile[kb0:kb1, kk, fb:fb + B],
                        ht[kb0:kb1, col:col + 1],
                        start=(i == 0),
                        stop=(i == last),
                        tile_position=(kb0, 0),
                    )
                # h_col = fp16(x - acc): one DVE op, no gpsimd cast
                nc.vector.tensor_sub(hcol, xt[:, k:k + 1], acc[:, :])
                if k == NR - 11:
                    # bulk of the output is final: overlap its DMA with the
                    # remaining rounds; only the last columns ship at the end
                    nc.scalar.dma_start(y_d[:, 0:dst + 1], ht[:, 0:dst + 1])

            # ---- output: ship the last columns; host reassembles
            nc.sync.dma_start(y_d[:, NR - 11 + OFFC + 1:NCOLS],
                              ht[:, NR - 11 + OFFC + 1:NCOLS])
    nc.compile()
    return nc


# --------------------------------------------------------------- entry point
_CACHE = {}


def kernel(delay_len_frames, raw_gain, raw_coeff_frames, excitation,
           exc_coefficients, n_samples):
    delay_len_frames = np.asarray(delay_len_frames, np.float32)
    raw_gain = np.asarray(raw_gain, np.float32)
    raw_coeff_frames = np.asarray(raw_coeff_frames, np.float32)
    excitation = np.asarray(excitation, np.float32)
    exc_coefficients = np.asarray(exc_coefficients, np.float32)
    assert int(n_samples) == T

    vf, s0 = _host_structure(delay_len_frames, raw_gain[0], raw_coeff_frames)
    plan, vtiles = _build_plan(vf, s0)

    vpack = np.ascontiguousarray(
        vtiles.astype(np.float16).transpose(1, 0, 2))   # [128, NR, VW]

    x = _lpc1(np.float64(excitation), np.float64(exc_coefficients[0, :, 0]))
    xp = np.zeros(TP, np.float32)
    xp[:T] = x.astype(np.float32)
    xcols = np.ascontiguousarray(xp.reshape(NR, B).T)   # [128, NR]

    key = hash((delay_len_frames.tobytes(), raw_gain.tobytes(),
                raw_coeff_frames.tobytes()))
    if key not in _CACHE:
        _CACHE[key] = (_build_kernel(plan), plan)
    nc, _ = _CACHE[key]

    in_map = dict(vtiles=np.ascontiguousarray(vpack), xcols=xcols)
    res = run_bass_kernel_spmd(nc, [in_map], core_ids=[0], trace=TRACE)
    if TRACE:
        global LAST_EXEC_NS, LAST_RES
        LAST_EXEC_NS = res.exec_time_ns
        LAST_RES = res
    y16 = res.results[0]["y16"]          # [128, NCOLS] fp16
    y = np.asarray(y16, np.float32)[:, OFFC:OFFC + NR]  # [128, NR]
    return np.ascontiguousarray(y.T.reshape(TP)[:T]).astype(np.float32)


if __name__ == "__main__":
    rng = np.random.default_rng(0)
    out = kernel(
        delay_len_frames=300 + 200 * rng.random(NFRAMES, np.float32),
        raw_gain=np.full(1, 2.5, np.float32),
        raw_coeff_frames=-2 * rng.random((NFRAMES, NCOEF), np.float32),
        excitation=rng.standard_normal(T).astype(np.float32),
        exc_coefficients=0.01 * rng.standard_normal((1, T, 1)).astype(np.float32),
        n_samples=T)
    print("kernel ran, out:", out.shape, out[:4])


# revision 41
# speedup vs baseline: 1.0167x; 1.0167x over previous
"""Trainium2 Bass kernel for nn_DiffKS (differentiable Karplus-Strong string).

Math:  y[t] = x[t] - sum_j vals[t,j] * y[t-1-z[t]-j],  z in [~289, ~517]
where x is the order-1-shaped excitation and vals/z come from a cubic-spline
upsampled delay/coefficient trajectory.

The feedback reaches >= ~290 samples back, so 128-sample blocks have no
intra-block dependency.  345 serial rounds; per round the 7-tap band (source
window ~134 wide, spanning 2 history columns, rarely 3) is evaluated as
exactly 2 (rarely 3) matmul pieces:

  piece A: K-range [a0,128) of history column c   (a0 in {0,64,96}, weights
           zero-padded below the first real source row)
  piece B: K-range [0,rB)   of history column c+1
  piece C: K-range [0,rC)   of history column c+2 (only ~4% of rounds;
           weights stored in the A-block's unused low rows)

All pieces use M=128 and PE-legal tile positions, PSUM-accumulated.  The
per-round chain is then just matmuls -> one DVE subtract that writes the new
fp16 history column directly (y = x - acc, cast on write).  No gpsimd cast,
no on-device output transpose: history is one [128, NCOLS] fp16 SBUF tile
(Tile hazards are range-granular, so per-column views don't false-serialize),
DMA'd out once; the host reassembles the final f32 signal.

The V tiles stream from DRAM in a partition-major layout ([128, NR, VW]) so
each partition reads contiguous multi-KB runs -- with the round-major layout
the DMA queues run at ~130 GB/s on 512B packets and gate the whole kernel.

Host does only the O(frames) spline prep, the integer structure plan, and
the (tiny) order-1 excitation scan.  Measured: ~94 us (baseline 144 us),
steady state is Tensor-engine LDWEIGHTS-throughput-bound (~102 ns per piece,
2.0 pieces/round, 96% PE occupancy); the serial feedback chain
(MM drain 173ns -> DVE 158ns, dependency distance ~2.27 rounds) sits just
under the PE throughput limit.
"""
import numpy as np
import ml_dtypes

import concourse.bacc as bacc
import concourse.mybir as mybir
from concourse.tile import TileContext
from concourse.bass_utils import run_bass_kernel_spmd

T = 44100
NFRAMES = 100
NCOEF = 6
B = 128
NR = (T + B - 1) // B          # 345 rounds
TP = NR * B                    # 44160
OFFC = 5                       # leading zero history columns
NCOLS = NR + OFFC              # 350
GRP = 16                       # V streaming group size
F32 = mybir.dt.float32
FP16 = mybir.dt.float16
NPH = 8                        # history phase tiles
SLOTS = (NCOLS + NPH - 1) // NPH   # 44
BOFF = 128                     # B block free-col offset
VW = 256                       # packed V tile free width (A cols 0..127, B cols 128..255)


TRACE = False
LAST_EXEC_NS = None
LAST_RES = None


# ----------------------------------------------------------------- host math
def _sigmoid(v):
    return 1.0 / (1.0 + np.exp(-v))


def _spline_eval(y, n_out):
    """Natural cubic spline on uniform knots in [0,1] (float64; the f32
    reference differs by ~1e-7 relative)."""
    n, d = y.shape
    h = 1.0 / (n - 1)
    rhs = 6.0 * (y[2:] - 2.0 * y[1:-1] + y[:-2]) / h
    Tm = (np.diag(np.full(n - 2, 4.0 * h))
          + np.diag(np.full(n - 3, h), 1)
          + np.diag(np.full(n - 3, h), -1))
    M_in = np.linalg.solve(Tm, rhs)
    M = np.concatenate([np.zeros((1, d)), M_in, np.zeros((1, d))])
    t_out = np.linspace(0.0, 1.0, n_out)
    idx = np.clip((t_out / h).astype(np.int32), 0, n - 2)
    f = (t_out - idx.astype(np.float64) * h)[:, None]
    y0, y1 = y[idx], y[idx + 1]
    M0, M1 = M[idx], M[idx + 1]
    b = (y1 - y0) / h - h * (2.0 * M0 + M1) / 6.0
    c = 0.5 * M0
    dd = (M1 - M0) / (6.0 * h)
    return y0 + f * (b + f * (c + f * dd))


def _host_structure(delay_len_frames, raw_gain, raw_coeff_frames):
    gain = _sigmoid(np.float64(raw_gain))
    sig = _sigmoid(np.float64(raw_coeff_frames))
    bf = sig / sig.sum(-1, keepdims=True) * gain
    params = np.concatenate([np.float64(delay_len_frames)[:, None], bf], axis=1)
    up = _spline_eval(params, T)
    delay, b = up[:, 0], up[:, 1:]
    z = np.floor(delay).astype(np.int64)
    alfa = delay - np.floor(delay)
    first = (-(1.0 - alfa) * b[:, 0])[:, None]
    mid = -(alfa[:, None] * b[:, :-1] + (1.0 - alfa)[:, None] * b[:, 1:])
    last = (-alfa * b[:, -1])[:, None]
    vals = np.concatenate([first, mid, last], axis=1)
    vf = vals[:, ::-1].copy()          # vf[t, jj] multiplies y[t-7-z[t]+jj]
    s0 = np.arange(T) - 7 - z
    return vf, s0


def _lpc1(e, a):
    """y[t] = e[t] - a[t]*y[t-1], vectorized by log-doubling in float64."""
    n = len(e)
    y = np.array(e, np.float64)
    c = -np.array(a, np.float64)       # y[t] = c[t]*y[t-1] + e[t]
    lag = 1
    while lag < n:
        y[lag:] = y[lag:] + c[lag:] * y[:-lag]
        c = np.concatenate([c[:lag] * 0.0, c[lag:] * c[:-lag]])
        lag *= 2
    return y


# ------------------------------------------------------------ blocked plan
def _build_plan(vf, s0):
    """Per-round packed V tiles [NR,128,VW] and piece lists."""
    s0p = np.concatenate([s0, s0[-1] + 1 + np.arange(TP - T)])
    vfp = np.concatenate([vf, np.zeros((TP - T, 7))]).astype(np.float64)
    soff = s0p + OFFC * B
    assert soff.min() >= 0
    vtiles = np.zeros((NR, B, VW), np.float64)
    plan = []
    for k in range(NR):
        t0 = k * B
        sl = soff[t0:t0 + B]
        smin = int(sl.min())
        smax = int(sl.max()) + 6
        c, r0 = divmod(smin, B)
        rB = smax - B * (c + 1) + 1
        rBc = min(max(rB, 0), B)
        rC = rB - B if rB > B else 0
        if rC:
            assert rC < r0, (k, rC, r0)
        a0 = 96 if r0 >= 96 else (64 if r0 >= 64 else 0)
        vt = vtiles[k]
        used = np.zeros((B, VW), np.bool_)
        for tt in range(B):
            base = int(sl[tt])
            for jj in range(7):
                s = base + jj
                row = s - B * c
                w = vfp[t0 + tt, jj]
                if row < B:
                    rr, cc = row, tt                # A block (col c)
                elif row < 2 * B:
                    rr, cc = row - B, BOFF + tt     # B block (col c+1)
                else:
                    assert row - 2 * B < r0
                    rr, cc = row - 2 * B, tt        # C in A block's low rows
                assert not used[rr, cc], (k, rr, cc)
                used[rr, cc] = True
                vt[rr, cc] = w
        pieces = []
        if c >= OFFC:                  # cols < OFFC are all-zero history
            pieces.append((a0, B, c, 0))
        if rBc and c + 1 >= OFFC:
            pieces.append((0, rBc, c + 1, BOFF))
        if rC and c + 2 >= OFFC:
            pieces.append((0, rC, c + 2, 0))
        plan.append(pieces)
    return plan, vtiles


# ------------------------------------------------------------- device build
def _build_kernel(plan):
    nc = bacc.Bacc("TRN2", target_bir_lowering=False, debug=False)
    # partition-major V layout: per-partition contiguous GRP*VW fp16 runs
    v_d = nc.dram_tensor("vtiles", [B, NR, VW], FP16, kind="ExternalInput")
    x_d = nc.dram_tensor("xcols", [B, NR], F32, kind="ExternalInput")
    y_d = nc.dram_tensor("y16", [B, NCOLS], FP16, kind="ExternalOutput")

    with TileContext(nc) as tc:
        with (
            tc.tile_pool(name="vpool", bufs=5) as vpool,
            tc.tile_pool(name="hpool", bufs=1) as hpool,
            tc.tile_pool(name="xpool", bufs=1) as xpool,
            tc.tile_pool(name="ps", bufs=6, space="PSUM") as ps,
        ):
            xt = xpool.tile([B, NR], F32)
            nc.sync.dma_start(xt[:, 0:32], x_d[:, 0:32])
            nc.scalar.dma_start(xt[:, 32:NR], x_d[:, 32:NR])
            ht = hpool.tile([B, NCOLS], FP16, tag="h", name="hist")
            nc.vector.memset(ht[:, 0:OFFC], 0.0)

            vtile = None
            for k in range(NR):
                g, kk = k // GRP, k % GRP
                if kk == 0:
                    gn = min(GRP, NR - g * GRP)
                    vtile = vpool.tile([B, GRP, VW], FP16, tag="v", name=f"v{g}")
                    g0 = g * GRP
                    if g == 0:
                        # split the first group finely across both queues:
                        # round 0 starts as soon as its own slice lands
                        splits = [(0, 1), (1, 2), (2, 4), (4, 8)]
                        if GRP > 8:
                            splits.append((8, GRP))
                        for h, (lo, hi) in enumerate(splits):
                            eng = nc.sync if (h % 2 == 0) else nc.scalar
                            eng.dma_start(
                                vtile[:, lo:hi, :], v_d[:, lo:hi, :])
                    elif g == 1:
                        # halve the second group too: avoids the early
                        # catch-up stall while the queues warm up
                        hgn = gn // 2
                        nc.scalar.dma_start(vtile[:, 0:hgn, :],
                                            v_d[:, g0:g0 + hgn, :])
                        nc.sync.dma_start(vtile[:, hgn:gn, :],
                                          v_d[:, g0 + hgn:g0 + gn, :])
                    else:
                        eng = nc.sync if (g % 2 == 0) else nc.scalar
                        eng.dma_start(
                            vtile[:, 0:gn, :],
                            v_d[:, g0:g0 + gn, :])
                pieces = plan[k]
                dst = k + OFFC
                hcol = ht[:, dst:dst + 1]
                if not pieces:
                    # all sources in zero history: y = x
                    nc.vector.tensor_copy(hcol, xt[:, k:k + 1])
                    continue
                acc = ps.tile([B, 1], F32, tag="acc", name=f"acc{k}")
                last = len(pieces) - 1
                for i, (kb0, kb1, col, fb) in enumerate(pieces):
                    nc.tensor.matmul(
                        acc[:, :],
                        vtile[kb0:kb1, kk, fb:fb + B],
                        ht[kb0:kb1, col:col + 1],
                        start=(i == 0),
                        stop=(i == last),
                        tile_position=(kb0, 0),
                    )
                # h_col = fp16(x - acc): one DVE op, no gpsimd cast
                nc.vector.tensor_sub(hcol, xt[:, k:k + 1], acc[:, :])
                if k == NR - 11:
                    # bulk of the output is final: overlap its DMA with the
                    # remaining rounds; only the last columns ship at the end
                    nc.scalar.dma_start(y_d[:, 0:dst + 1], ht[:, 0:dst + 1])

            # ---- output: ship the last columns; host reassembles
            nc.sync.dma_start(y_d[:, NR - 11 + OFFC + 1:NCOLS],
                              ht[:, NR - 11 + OFFC + 1:NCOLS])
    nc.compile()
    return nc


# --------------------------------------------------------------- entry point
_CACHE = {}


def kernel(delay_len_frames, raw_gain, raw_coeff_frames, excitation,
           exc_coefficients, n_samples):
    delay_len_frames = np.asarray(delay_len_frames, np.float32)
    raw_gain = np.asarray(raw_gain, np.float32)
    raw_coeff_frames = np.asarray(raw_coeff_frames, np.float32)
    excitation = np.asarray(excitation, np.float32)
    exc_coefficients = np.asarray(exc_coefficients, np.float32)
    assert int(n_samples) == T

    vf, s0 = _host_structure(delay_len_frames, raw_gain[0], raw_coeff_frames)
    plan, vtiles = _build_plan(vf, s0)

    vpack = np.ascontiguousarray(
        vtiles.astype(np.float16).transpose(1, 0, 2))   # [128, NR, VW]

    x = _lpc1(np.float64(excitation), np.float64(exc_coefficients[0, :, 0]))
    xp = np.zeros(TP, np.float32)
    xp[:T] = x.astype(np.float32)
    xcols = np.ascontiguousarray(xp.reshape(NR, B).T)   # [128, NR]

    key = hash((delay_len_frames.tobytes(), raw_gain.tobytes(),
                raw_coeff_frames.tobytes()))
    if key not in _CACHE:
        _CACHE[key] = (_build_kernel(plan), plan)
    nc, _ = _CACHE[key]

    in_map = dict(vtiles=np.ascontiguousarray(vpack), xcols=xcols)
    res = run_bass_kernel_spmd(nc, [in_map], core_ids=[0], trace=TRACE)
    if TRACE:
        global LAST_EXEC_NS, LAST_RES
        LAST_EXEC_NS = res.exec_time_ns
        LAST_RES = res
    y16 = res.results[0]["y16"]          # [128, NCOLS] fp16
    y = np.asarray(y16, np.float32)[:, OFFC:OFFC + NR]  # [128, NR]
    return np.ascontiguousarray(y.T.reshape(TP)[:T]).astype(np.float32)


if __name__ == "__main__":
    rng = np.random.default_rng(0)
    out = kernel(
        delay_len_frames=300 + 200 * rng.random(NFRAMES, np.float32),
        raw_gain=np.full(1, 2.5, np.float32),
        raw_coeff_frames=-2 * rng.random((NFRAMES, NCOEF), np.float32),
        excitation=rng.standard_normal(T).astype(np.float32),
        exc_coefficients=0.01 * rng.standard_normal((1, T, 1)).astype(np.float32),
        n_samples=T)
    print("kernel ran, out:", out.shape, out[:4])


# revision 42
# speedup vs baseline: 1.0430x; 1.0258x over previous
"""Trainium2 Bass kernel for nn_DiffKS (differentiable Karplus-Strong string).

Math:  y[t] = x[t] - sum_j vals[t,j] * y[t-1-z[t]-j],  z in [~289, ~517]
where x is the order-1-shaped excitation and vals/z come from a cubic-spline
upsampled delay/coefficient trajectory.

The feedback reaches >= ~290 samples back, so 128-sample blocks have no
intra-block dependency.  345 serial rounds; per round the 7-tap band (source
window ~134 wide, spanning 2 history columns, rarely 3) is evaluated as
exactly 2 (rarely 3) matmul pieces:

  piece A: K-range [a0,128) of history column c   (a0 in {0,64,96}, weights
           zero-padded below the first real source row)
  piece B: K-range [0,rB)   of history column c+1
  piece C: K-range [0,rC)   of history column c+2 (only ~4% of rounds;
           weights stored in the A-block's unused low rows)

All pieces use M=128 and PE-legal tile positions, PSUM-accumulated.  The
per-round chain is then just matmuls -> one DVE subtract that writes the new
fp16 history column directly (y = x - acc, cast on write).  No gpsimd cast,
no on-device output transpose: history is one [128, NCOLS] fp16 SBUF tile
(Tile hazards are range-granular, so per-column views don't false-serialize),
DMA'd out once; the host reassembles the final f32 signal.

The V tiles stream from DRAM in a partition-major layout ([128, NR, VW]) so
each partition reads contiguous multi-KB runs -- with the round-major layout
the DMA queues run at ~130 GB/s on 512B packets and gate the whole kernel.

Host does only the O(frames) spline prep, the integer structure plan, and
the (tiny) order-1 excitation scan.  Measured: ~94 us (baseline 144 us),
steady state is Tensor-engine LDWEIGHTS-throughput-bound (~102 ns per piece,
2.0 pieces/round, 96% PE occupancy); the serial feedback chain
(MM drain 173ns -> DVE 158ns, dependency distance ~2.27 rounds) sits just
under the PE throughput limit.
"""
import numpy as np
import ml_dtypes

import concourse.bacc as bacc
import concourse.mybir as mybir
from concourse.tile import TileContext
from concourse.bass_utils import run_bass_kernel_spmd

T = 44100
NFRAMES = 100
NCOEF = 6
B = 128
NR = (T + B - 1) // B          # 345 rounds
TP = NR * B                    # 44160
OFFC = 5                       # leading zero history columns
NCOLS = NR + OFFC              # 350
GRP = 16                       # V streaming group size
F32 = mybir.dt.float32
FP16 = mybir.dt.float16
NPH = 8                        # history phase tiles
SLOTS = (NCOLS + NPH - 1) // NPH   # 44
BOFF = 128                     # B block free-col offset
VW = 256                       # packed V tile free width (A cols 0..127, B cols 128..255)


TRACE = False
LAST_EXEC_NS = None
LAST_RES = None


# ----------------------------------------------------------------- host math
def _sigmoid(v):
    return 1.0 / (1.0 + np.exp(-v))


def _spline_eval(y, n_out):
    """Natural cubic spline on uniform knots in [0,1] (float64; the f32
    reference differs by ~1e-7 relative)."""
    n, d = y.shape
    h = 1.0 / (n - 1)
    rhs = 6.0 * (y[2:] - 2.0 * y[1:-1] + y[:-2]) / h
    Tm = (np.diag(np.full(n - 2, 4.0 * h))
          + np.diag(np.full(n - 3, h), 1)
          + np.diag(np.full(n - 3, h), -1))
    M_in = np.linalg.solve(Tm, rhs)
    M = np.concatenate([np.zeros((1, d)), M_in, np.zeros((1, d))])
    t_out = np.linspace(0.0, 1.0, n_out)
    idx = np.clip((t_out / h).astype(np.int32), 0, n - 2)
    f = (t_out - idx.astype(np.float64) * h)[:, None]
    y0, y1 = y[idx], y[idx + 1]
    M0, M1 = M[idx], M[idx + 1]
    b = (y1 - y0) / h - h * (2.0 * M0 + M1) / 6.0
    c = 0.5 * M0
    dd = (M1 - M0) / (6.0 * h)
    return y0 + f * (b + f * (c + f * dd))


def _host_structure(delay_len_frames, raw_gain, raw_coeff_frames):
    gain = _sigmoid(np.float64(raw_gain))
    sig = _sigmoid(np.float64(raw_coeff_frames))
    bf = sig / sig.sum(-1, keepdims=True) * gain
    params = np.concatenate([np.float64(delay_len_frames)[:, None], bf], axis=1)
    up = _spline_eval(params, T)
    delay, b = up[:, 0], up[:, 1:]
    z = np.floor(delay).astype(np.int64)
    alfa = delay - np.floor(delay)
    first = (-(1.0 - alfa) * b[:, 0])[:, None]
    mid = -(alfa[:, None] * b[:, :-1] + (1.0 - alfa)[:, None] * b[:, 1:])
    last = (-alfa * b[:, -1])[:, None]
    vals = np.concatenate([first, mid, last], axis=1)
    vf = vals[:, ::-1].copy()          # vf[t, jj] multiplies y[t-7-z[t]+jj]
    s0 = np.arange(T) - 7 - z
    return vf, s0


def _lpc1(e, a):
    """y[t] = e[t] - a[t]*y[t-1], vectorized by log-doubling in float64."""
    n = len(e)
    y = np.array(e, np.float64)
    c = -np.array(a, np.float64)       # y[t] = c[t]*y[t-1] + e[t]
    lag = 1
    while lag < n:
        y[lag:] = y[lag:] + c[lag:] * y[:-lag]
        c = np.concatenate([c[:lag] * 0.0, c[lag:] * c[:-lag]])
        lag *= 2
    return y


# ------------------------------------------------------------ blocked plan
def _build_plan(vf, s0):
    """Per-round packed V tiles [NR,128,VW] and piece lists."""
    s0p = np.concatenate([s0, s0[-1] + 1 + np.arange(TP - T)])
    vfp = np.concatenate([vf, np.zeros((TP - T, 7))]).astype(np.float64)
    soff = s0p + OFFC * B
    assert soff.min() >= 0
    vtiles = np.zeros((NR, B, VW), np.float64)
    plan = []
    for k in range(NR):
        t0 = k * B
        sl = soff[t0:t0 + B]
        smin = int(sl.min())
        smax = int(sl.max()) + 6
        c, r0 = divmod(smin, B)
        rB = smax - B * (c + 1) + 1
        rBc = min(max(rB, 0), B)
        rC = rB - B if rB > B else 0
        if rC:
            assert rC < r0, (k, rC, r0)
        a0 = 96 if r0 >= 96 else (64 if r0 >= 64 else 0)
        vt = vtiles[k]
        used = np.zeros((B, VW), np.bool_)
        for tt in range(B):
            base = int(sl[tt])
            for jj in range(7):
                s = base + jj
                row = s - B * c
                w = vfp[t0 + tt, jj]
                if row < B:
                    rr, cc = row, tt                # A block (col c)
                elif row < 2 * B:
                    rr, cc = row - B, BOFF + tt     # B block (col c+1)
                else:
                    assert row - 2 * B < r0
                    rr, cc = row - 2 * B, tt        # C in A block's low rows
                assert not used[rr, cc], (k, rr, cc)
                used[rr, cc] = True
                vt[rr, cc] = w
        pieces = []
        if c >= OFFC:                  # cols < OFFC are all-zero history
            pieces.append((a0, B, c, 0))
        if rBc and c + 1 >= OFFC:
            pieces.append((0, rBc, c + 1, BOFF))
        if rC and c + 2 >= OFFC:
            pieces.append((0, rC, c + 2, 0))
        plan.append(pieces)
    return plan, vtiles


# ------------------------------------------------------------- device build
def _build_kernel(plan):
    nc = bacc.Bacc("TRN2", target_bir_lowering=False, debug=False)
    # partition-major V layout: per-partition contiguous GRP*VW fp16 runs
    v_d = nc.dram_tensor("vtiles", [B, NR, VW], FP16, kind="ExternalInput")
    x_d = nc.dram_tensor("xcols", [B, NR], F32, kind="ExternalInput")
    y_d = nc.dram_tensor("y16", [B, NCOLS], FP16, kind="ExternalOutput")

    with TileContext(nc) as tc:
        with (
            tc.tile_pool(name="vpool", bufs=5) as vpool,
            tc.tile_pool(name="hpool", bufs=1) as hpool,
            tc.tile_pool(name="xpool", bufs=1) as xpool,
            tc.tile_pool(name="ps", bufs=6, space="PSUM") as ps,
        ):
            xt = xpool.tile([B, NR], F32)
            nc.sync.dma_start(xt[:, 0:32], x_d[:, 0:32])
            nc.scalar.dma_start(xt[:, 32:NR], x_d[:, 32:NR])
            ht = hpool.tile([B, NCOLS], FP16, tag="h", name="hist")
            nc.vector.memset(ht[:, 0:OFFC], 0.0)

            vtile = None
            for k in range(NR):
                g, kk = k // GRP, k % GRP
                if kk == 0:
                    gn = min(GRP, NR - g * GRP)
                    vtile = vpool.tile([B, GRP, VW], FP16, tag="v", name=f"v{g}")
                    g0 = g * GRP
                    if g == 0:
                        # split the first group finely across both queues:
                        # round 0 starts as soon as its own slice lands
                        splits = [(0, 1), (1, 2), (2, 4), (4, 8)]
                        if GRP > 8:
                            splits.append((8, GRP))
                        for h, (lo, hi) in enumerate(splits):
                            eng = nc.sync if (h % 2 == 0) else nc.scalar
                            eng.dma_start(
                                vtile[:, lo:hi, :], v_d[:, lo:hi, :])
                    elif g in (1, 2, 3):
                        # halve the warm-up groups across both queues:
                        # avoids catch-up stalls while the DMA ramps
                        hgn = gn // 2
                        nc.scalar.dma_start(vtile[:, 0:hgn, :],
                                            v_d[:, g0:g0 + hgn, :])
                        nc.sync.dma_start(vtile[:, hgn:gn, :],
                                          v_d[:, g0 + hgn:g0 + gn, :])
                    else:
                        eng = nc.sync if (g % 2 == 0) else nc.scalar
                        eng.dma_start(
                            vtile[:, 0:gn, :],
                            v_d[:, g0:g0 + gn, :])
                pieces = plan[k]
                dst = k + OFFC
                hcol = ht[:, dst:dst + 1]
                if not pieces:
                    # all sources in zero history: y = x
                    nc.vector.tensor_copy(hcol, xt[:, k:k + 1])
                    continue
                acc = ps.tile([B, 1], F32, tag="acc", name=f"acc{k}")
                last = len(pieces) - 1
                for i, (kb0, kb1, col, fb) in enumerate(pieces):
                    nc.tensor.matmul(
                        acc[:, :],
                        vtile[kb0:kb1, kk, fb:fb + B],
                        ht[kb0:kb1, col:col + 1],
                        start=(i == 0),
                        stop=(i == last),
                        tile_position=(kb0, 0),
                    )
                # h_col = fp16(x - acc): one DVE op, no gpsimd cast
                nc.vector.tensor_sub(hcol, xt[:, k:k + 1], acc[:, :])
                if k == NR - 11:
                    # bulk of the output is final: overlap its DMA with the
                    # remaining rounds; only the last columns ship at the end
                    nc.scalar.dma_start(y_d[:, 0:dst + 1], ht[:, 0:dst + 1])

            # ---- output: ship the last columns; host reassembles
            nc.sync.dma_start(y_d[:, NR - 11 + OFFC + 1:NCOLS],
                              ht[:, NR - 11 + OFFC + 1:NCOLS])
    nc.compile()
    return nc


# --------------------------------------------------------------- entry point
_CACHE = {}


def kernel(delay_len_frames, raw_gain, raw_coeff_frames, excitation,
           exc_coefficients, n_samples):
    delay_len_frames = np.asarray(delay_len_frames, np.float32)
    raw_gain = np.asarray(raw_gain, np.float32)
    raw_coeff_frames = np.asarray(raw_coeff_frames, np.float32)
    excitation = np.asarray(excitation, np.float32)
    exc_coefficients = np.asarray(exc_coefficients, np.float32)
    assert int(n_samples) == T

    vf, s0 = _host_structure(delay_len_frames, raw_gain[0], raw_coeff_frames)
    plan, vtiles = _build_plan(vf, s0)

    vpack = np.ascontiguousarray(
        vtiles.astype(np.float16).transpose(1, 0, 2))   # [128, NR, VW]

    x = _lpc1(np.float64(excitation), np.float64(exc_coefficients[0, :, 0]))
    xp = np.zeros(TP, np.float32)
    xp[:T] = x.astype(np.float32)
    xcols = np.ascontiguousarray(xp.reshape(NR, B).T)   # [128, NR]

    key = hash((delay_len_frames.tobytes(), raw_gain.tobytes(),
                raw_coeff_frames.tobytes()))
    if key not in _CACHE:
        _CACHE[key] = (_build_kernel(plan), plan)
    nc, _ = _CACHE[key]

    in_map = dict(vtiles=np.ascontiguousarray(vpack), xcols=xcols)
    res = run_bass_kernel_spmd(nc, [in_map], core_ids=[0], trace=TRACE)
    if TRACE:
        global LAST_EXEC_NS, LAST_RES
        LAST_EXEC_NS = res.exec_time_ns
        LAST_RES = res
    y16 = res.results[0]["y16"]          # [128, NCOLS] fp16
    y = np.asarray(y16, np.float32)[:, OFFC:OFFC + NR]  # [128, NR]
    return np.ascontiguousarray(y.T.reshape(TP)[:T]).astype(np.float32)


if __name__ == "__main__":
    rng = np.random.default_rng(0)
    out = kernel(
        delay_len_frames=300 + 200 * rng.random(NFRAMES, np.float32),
        raw_gain=np.full(1, 2.5, np.float32),
        raw_coeff_frames=-2 * rng.random((NFRAMES, NCOEF), np.float32),
        excitation=rng.standard_normal(T).astype(np.float32),
        exc_coefficients=0.01 * rng.standard_normal((1, T, 1)).astype(np.float32),
        n_samples=T)
    print("kernel ran, out:", out.shape, out[:4])
